# revision 18
# baseline (speedup 1.0000x reference)
"""Trainium2 Bass kernel for nn_CMVNet (moe_routing).

Reference computation:
    h = relu(x @ W1.T + b1)            # [N, HID]
    e = c[num]                         # [N] per-sample expert index
    y = einsum('noh,nh->no', We[e], h) + be[e]   # OUT=1
    out = sigmoid(y)                   # [N, 1]

Data-parallel over 8 cores (N/8 = 16384 rows each). Two device programs:

v3 (default): host sorts each shard's tokens by expert. Stage A runs in
  fp8 (e4m3) with perf_mode=DoubleRow: one matmul contracts all 256 input
  features (2 fp8 weights per PE cell), ~1.8x the bf16 rate. Host feeds
  x*16 and W1*16 in e4m3 so PSUM holds 256*(x@W1.T); the relu drain adds
  256*b1 and writes h*256 in bf16. Chunks are processed in PAIRS sharing
  a 2-bank [128, 1024] PSUM tile so each relu/bias op covers 1024 columns
  (amortizes the fixed ACT/DVE overhead); ops are split ACT/DVE to
  balance the two engines (both run ~1 elem/lane/cycle from fp32 PSUM).
  Stage B (per-sample expert dot) stays bf16: each 512-token block of the
  sorted order spans <= 8 distinct experts; 4 blocks are computed
  CONCURRENTLY (one per 32-column strip of the PE array via
  tile_position) against per-block expert slot tables (We/256 in bf16).
      select:  masked = (ps + beB) * ohB  (one fused DVE op),
               y4 = I4.T @ masked   ([4, 512] matvec on TensorE),
               sigmoid on ACT, DMA out, host un-permutes.
v1 (fallback, if any block spans > 8 experts): dense float32r scores
  against all 100 experts + one-hot select.

Quantization error (vs fp32 reference) is ~1.4e-2 relative L2, within
the 2e-2 gate; PSUM accumulation is fp32 throughout.
"""

import numpy as np

N, D_IN, HID, OUT, E = 131072, 256, 1024, 1, 100
NCORES = 8
NSH = N // NCORES          # 16384 rows per core
EP = 128                   # experts padded to full partition dim (v1)
CHUNK = 512
D_T = D_IN // 128          # 2 contraction k-tiles for stage A
H_T = HID // 128           # 8 hid tiles
NBLK = NSH // CHUNK        # 32 sorted-token blocks per core
STRIPS = 4                 # concurrent col-strips on the PE array
SLOTS = 32                 # on-chip expert slots per block
SLOTS_DMA = 8              # slots actually stored/DMAd per block
GROUPS = NBLK // STRIPS    # 8 strip-groups per core
SX = 16.0                  # fp8 input scale (x*SX in e4m3)
SW = 16.0                  # fp8 weight scale (W1*SW in e4m3)
SH = SX * SW               # h leaves stage A scaled by SH (bf16)

TRACE = False              # set by test harness for profiled runs
LAST_RESULTS = None        # BassKernelResults of the last run (for test.py)

_BUILT = {}                # (version, nsh) -> compiled Bass module


def _bf16_dt():
    import ml_dtypes
    return ml_dtypes.bfloat16


def _fp8_dt():
    import ml_dtypes
    return ml_dtypes.float8_e4m3   # TRN FP8_EXP4 (IEEE-ish, max +-240)


def _tf32_round(a):
    """Round fp32 ndarray to TF32 (10-bit mantissa), round-to-nearest-even."""
    u = np.ascontiguousarray(a, dtype=np.float32).view(np.uint32)
    u = (u + 0x00000FFF + ((u >> 13) & 1)) & np.uint32(0xFFFFE000)
    return u.view(np.float32)


def _mk_bass(nsh):
    from concourse import bacc
    return bacc.Bacc("TRN2", target_bir_lowering=False, debug=False)


# --------------------------------------------------------------------------
# v1: dense scores against all experts + one-hot select (fallback)
# --------------------------------------------------------------------------
def _build_nc_v1(nsh):
    from contextlib import ExitStack

    import concourse.mybir as mybir
    import concourse.tile as tile

    fp32 = mybir.dt.float32
    fr = mybir.dt.float32r
    AF = mybir.ActivationFunctionType
    OP = mybir.AluOpType

    nchunk = nsh // CHUNK
    nc = _mk_bass(nsh)

    xT = nc.dram_tensor("xT", [D_IN, nsh], fr, kind="ExternalInput")
    w1T = nc.dram_tensor("w1T", [D_IN, HID], fr, kind="ExternalInput")
    b1c = nc.dram_tensor("b1c", [128, H_T], fp32, kind="ExternalInput")
    weT = nc.dram_tensor("weT", [HID, EP], fr, kind="ExternalInput")
    bec = nc.dram_tensor("bec", [EP, 1], fp32, kind="ExternalInput")
    oh = nc.dram_tensor("oh", [EP, nsh], fp32, kind="ExternalInput")
    y = nc.dram_tensor("y", [1, nsh], fp32, kind="ExternalOutput")

    xT_v = xT.rearrange("(c p) n -> p c n", p=128)
    w1T_v = w1T.rearrange("(c p) h -> p c h", p=128)
    weT_v = weT.rearrange("(k p) e -> p k e", p=128)

    with tile.TileContext(nc) as tc, ExitStack() as ctx:
        cpool = ctx.enter_context(tc.tile_pool(name="consts", bufs=1))
        xin = ctx.enter_context(tc.tile_pool(name="xin", bufs=3))
        ohin = ctx.enter_context(tc.tile_pool(name="ohin", bufs=3))
        hpool = ctx.enter_context(tc.tile_pool(name="h", bufs=2))
        mpool = ctx.enter_context(tc.tile_pool(name="masked", bufs=3))
        ypool = ctx.enter_context(tc.tile_pool(name="yrow", bufs=3))
        php = ctx.enter_context(tc.tile_pool(name="ph", bufs=3, space="PSUM"))
        psp = ctx.enter_context(tc.tile_pool(name="ps", bufs=2, space="PSUM"))
        pyp = ctx.enter_context(tc.tile_pool(name="py", bufs=2, space="PSUM"))

        w1_sb = cpool.tile([128, D_T, HID], fr)
        we_sb = cpool.tile([128, H_T, EP], fr)
        b1_sb = cpool.tile([128, H_T], fp32)
        be_sb = cpool.tile([128, 1], fp32)
        ones_f32 = cpool.tile([128, 1], fp32)
        ones_sb = cpool.tile([128, 1], fr)
        nc.sync.dma_start(w1_sb[:], w1T_v[:])
        nc.sync.dma_start(we_sb[:], weT_v[:])
        nc.sync.dma_start(b1_sb[:], b1c[:])
        nc.sync.dma_start(be_sb[:], bec[:])
        nc.vector.memset(ones_f32[:], 1.0)
        nc.vector.tensor_copy(ones_sb[:], ones_f32[:])

        for ci in range(nchunk):
            n0 = ci * CHUNK
            xts = xin.tile([128, D_T, CHUNK], fr)
            nc.sync.dma_start(xts[:], xT_v[:, :, n0:n0 + CHUNK])
            oh_sb = ohin.tile([128, CHUNK], fp32)
            nc.sync.dma_start(oh_sb[:], oh[:, n0:n0 + CHUNK])

            hT = hpool.tile([128, H_T, CHUNK], fr)
            phs = []
            for j in range(H_T):
                ph = php.tile([128, CHUNK], fp32)
                for c in range(D_T):
                    nc.tensor.matmul(
                        ph[:],
                        w1_sb[:, c, 128 * j:128 * (j + 1)],
                        xts[:, c, :],
                        start=(c == 0),
                        stop=(c == D_T - 1),
                    )
                phs.append(ph)
            for j in range(H_T):
                if j % 2 == 0:
                    nc.scalar.activation(
                        hT[:, j, :], phs[j][:], AF.Relu,
                        bias=b1_sb[:, j:j + 1], scale=1.0,
                    )
                else:
                    nc.vector.tensor_scalar(
                        hT[:, j, :], phs[j][:],
                        b1_sb[:, j:j + 1], 0.0,
                        OP.add, OP.max,
                    )

            ps = psp.tile([128, CHUNK], fp32)
            for j in range(H_T):
                nc.tensor.matmul(
                    ps[:],
                    we_sb[:, j, :],
                    hT[:, j, :],
                    start=(j == 0),
                    stop=(j == H_T - 1),
                )

            masked = mpool.tile([128, CHUNK], fr)
            nc.vector.scalar_tensor_tensor(
                masked[:], ps[:], be_sb[:, 0:1], oh_sb[:],
                OP.add, OP.mult,
            )

            py = pyp.tile([1, CHUNK], fp32)
            nc.tensor.matmul(py[:], ones_sb[:], masked[:], start=True, stop=True)
            y_sb = ypool.tile([1, CHUNK], fp32)
            nc.scalar.activation(y_sb[:], py[:], AF.Sigmoid)
            nc.sync.dma_start(y[0:1, n0:n0 + CHUNK], y_sb[:])

    nc.compile()
    return nc


# --------------------------------------------------------------------------
# v3: fp8 DoubleRow stage A, paired-chunk wide relu, bf16 strip stage B
# --------------------------------------------------------------------------
def _build_nc_v3(nsh):
    from contextlib import ExitStack

    import concourse.mybir as mybir
    import concourse.tile as tile

    fp32 = mybir.dt.float32
    fr = mybir.dt.float32r
    bf16 = mybir.dt.bfloat16
    f8 = mybir.dt.float8e4
    AF = mybir.ActivationFunctionType
    OP = mybir.AluOpType
    DR = mybir.MatmulPerfMode.DoubleRow

    nblk = nsh // CHUNK
    groups = nblk // STRIPS
    PAIR = 2 * CHUNK
    nc = _mk_bass(nsh)

    xT = nc.dram_tensor("xT", [D_IN, nsh], f8, kind="ExternalInput")
    w1T = nc.dram_tensor("w1T", [D_IN, HID], f8, kind="ExternalInput")
    b1c = nc.dram_tensor("b1c", [128, H_T], fp32, kind="ExternalInput")
    web = nc.dram_tensor("web", [HID, nblk * SLOTS], bf16,
                         kind="ExternalInput")
    beh = nc.dram_tensor("beh", [128, groups], fp32, kind="ExternalInput")
    ohb = nc.dram_tensor("ohb", [128, groups * CHUNK], bf16,
                         kind="ExternalInput")
    i4 = nc.dram_tensor("i4", [128, STRIPS], fr, kind="ExternalInput")
    y = nc.dram_tensor("y", [STRIPS, groups * CHUNK], fp32,
                       kind="ExternalOutput")

    xT_v = xT.rearrange("(c p) n -> p c n", p=128)
    w1T_v = w1T.rearrange("(c p) h -> p c h", p=128)
    web_v = web.rearrange("(k p) (b s) -> p k b s", p=128, s=SLOTS)

    with tile.TileContext(nc) as tc, ExitStack() as ctx:
        cpool = ctx.enter_context(tc.tile_pool(name="consts", bufs=1))
        xin = ctx.enter_context(tc.tile_pool(name="xin", bufs=4))
        ohin = ctx.enter_context(tc.tile_pool(name="ohin", bufs=2))
        hpool = ctx.enter_context(tc.tile_pool(name="h", bufs=4))
        mpool = ctx.enter_context(tc.tile_pool(name="masked", bufs=2))
        ypool = ctx.enter_context(tc.tile_pool(name="yrow", bufs=2))
        # ALL of PSUM is one pool: 4 slots x 2 banks. Stage-A fp32 tiles
        # rotate through it; the stage-B score tile, the select output and
        # the warmup target borrow slots transiently. 4-deep buffering is
        # what keeps the relu drains (the bottleneck engines) saturated
        # across the ~100ns semaphore hops of the fill->drain->refill loop.
        php = ctx.enter_context(tc.tile_pool(name="ph", bufs=4, space="PSUM"))

        w1_sb = cpool.tile([128, D_T, HID], f8)
        web_sb = cpool.tile([128, H_T, nblk, SLOTS], bf16)
        b1_sb = cpool.tile([128, H_T], fp32)
        beh_sb = cpool.tile([128, groups], fp32)
        i4_sb = cpool.tile([128, STRIPS], fr)
        # Cold DMA completions serialize at ~2us apiece per queue, so the
        # head spreads the gating transfers across queues: x rides sync
        # (+scalar for the second half), w1/b1 ride the vector queue, the
        # slot table + small consts the gpsimd queue.
        nc.scalar.dma_start(w1_sb[:, :, 0:128], w1T_v[:, :, 0:128])
        nc.scalar.dma_start(b1_sb[:], b1c[:])
        nc.gpsimd.dma_start(web_sb[:], web_v[:])
        nc.gpsimd.dma_start(beh_sb[:], beh[:])
        nc.gpsimd.dma_start(i4_sb[:], i4[:])
        # warm the ACT sigmoid table during the idle head (the first real
        # sigmoid otherwise pays a 1.3us mid-stream ACT_TABLE_LOAD stall)
        warm_sb = cpool.tile([1, 2], fp32)
        nc.gpsimd.memset(warm_sb[:], 0.0)
        nc.scalar.activation(warm_sb[:], warm_sb[:], AF.Sigmoid)
        # warm the PE HAM clock gate during the DMA head with FULL-ARRAY
        # zero matmuls (no DMA dependency). Tiny 8-col warmups give almost
        # no cycle credit and leave the PE at the MID p-state (630ns DR
        # matmuls) deep into the kernel; [128x128]x512 ones accumulate
        # real busy cycles. Sized to end roughly when the first x lands.
        wmm = cpool.tile([128, CHUNK], bf16)
        nc.gpsimd.memset(wmm[:], 0.0)
        wps = php.tile([128, CHUNK], fp32, tag="ph")
        for _ in range(6):
            nc.tensor.matmul(wps[:], wmm[:, 0:128], wmm[:],
                             start=True, stop=True)

        def emit_select(sg, smasked, split=1):
            # y4 = I4.T @ masked, sigmoid, DMA out. Emitted one group late
            # so the PE never waits on the DVE mask op.
            w = CHUNK // split
            for hh in range(split):
                h0 = hh * w
                py = php.tile([STRIPS, w], fp32, tag="ph")
                nc.tensor.matmul(py[:], i4_sb[:], smasked[:, h0:h0 + w],
                                 start=True, stop=True)
                y_sb = ypool.tile([STRIPS, w], fp32, tag="ysb")
                nc.scalar.activation(y_sb[:], py[:], AF.Sigmoid)
                nc.sync.dma_start(
                    y[:, sg * CHUNK + h0:sg * CHUNK + h0 + w], y_sb[:])

        pending_select = None
        for g in range(groups):
            hts = []
            for lp in range(2):
                p = 2 * g + lp
                n0 = p * PAIR
                xts = xin.tile([128, D_T, PAIR], f8, tag="xts")
                if p == 0:
                    # first pair split across two queues so both halves
                    # complete their (serial, ~2us) cold DMA in parallel
                    nc.sync.dma_start(xts[:, :, 0:CHUNK],
                                      xT_v[:, :, n0:n0 + CHUNK])
                    nc.scalar.dma_start(xts[:, :, CHUNK:PAIR],
                                        xT_v[:, :, n0 + CHUNK:n0 + PAIR])
                    nc.scalar.dma_start(w1_sb[:, :, 128:HID],
                                        w1T_v[:, :, 128:HID])
                else:
                    nc.sync.dma_start(xts[:], xT_v[:, :, n0:n0 + PAIR])
                if lp == 0:
                    # prefetch the one-hot select mask for this group
                    oh_sb = ohin.tile([128, CHUNK], bf16)
                    nc.sync.dma_start(
                        oh_sb[:], ohb[:, g * CHUNK:(g + 1) * CHUNK])

                hT = hpool.tile([128, H_T, PAIR], bf16, tag="hT")
                for k in range(H_T):
                    ph = php.tile([128, PAIR], fp32, tag="ph")
                    for hh in range(2):
                        nc.tensor.matmul(
                            ph[:, hh * CHUNK:(hh + 1) * CHUNK],
                            w1_sb[:, :, 128 * k:128 * (k + 1)],
                            xts[:, :, hh * CHUNK:(hh + 1) * CHUNK],
                            start=True,
                            stop=True,
                            perf_mode=DR,
                        )
                    # relu+bias drain, split ACT/DVE ~4.25/3.75 (measured:
                    # ACT ~1.12us, DVE ~1.28us per 1024-wide op; DVE also
                    # owns the mask op, ACT the sigmoids)
                    on_act = (k % 2 == 0) or (
                        k == 7 and lp == 1 and g % 2 == 0)
                    if on_act:
                        nc.scalar.activation(
                            hT[:, k, :], ph[:], AF.Relu,
                            bias=b1_sb[:, k:k + 1], scale=1.0,
                        )
                    else:
                        nc.vector.tensor_scalar(
                            hT[:, k, :], ph[:],
                            b1_sb[:, k:k + 1], 0.0,
                            OP.add, OP.max,
                        )
                    if lp == 1 and k == 0 and pending_select is not None:
                        emit_select(*pending_select)
                        pending_select = None
                hts.append(hT)

            # stage B: 4 blocks concurrently, one per 32-col strip (bf16);
            # the score tile borrows a rotating PSUM slot. While the PE
            # runs these 8 k-steps the drains chew through their 4-deep
            # backlog, so they never idle.
            ps = php.tile([128, CHUNK], fp32, tag="ph")
            for k in range(H_T):
                for j in range(STRIPS):
                    blk = g * STRIPS + j
                    pair, off = j // 2, (j % 2) * CHUNK
                    nc.tensor.matmul(
                        ps[32 * j:32 * (j + 1), :],
                        web_sb[:, k, blk, :],
                        hts[pair][:, k, off:off + CHUNK],
                        start=(k == 0),
                        stop=(k == H_T - 1),
                        tile_position=(0, 32 * j),
                        skip_group_check=True,
                    )
            masked = mpool.tile([128, CHUNK], fr)
            nc.vector.scalar_tensor_tensor(
                masked[:], ps[:], beh_sb[:, g:g + 1], oh_sb[:],
                OP.add, OP.mult,
            )
            if g == groups - 1:
                # final select split in halves so DVE/PE/ACT/DMA pipeline
                # in the kernel tail instead of serializing full-width
                emit_select(g, masked, split=2)
            else:
                pending_select = (g, masked)

    nc.compile()
    return nc


def _get_nc(version, nsh=NSH):
    key = (version, nsh)
    if key not in _BUILT:
        _BUILT[key] = (_build_nc_v3 if version == 3 else _build_nc_v1)(nsh)
    return _BUILT[key]


# --------------------------------------------------------------------------
# host prep
# --------------------------------------------------------------------------
def _common_prep(x, W1, b1, We, be, num, c):
    x = np.asarray(x, dtype=np.float32)
    W1 = np.asarray(W1, dtype=np.float32)
    b1 = np.asarray(b1, dtype=np.float32)
    We = np.asarray(We, dtype=np.float32)
    be = np.asarray(be, dtype=np.float32)
    eidx = np.asarray(np.asarray(c)[np.asarray(num)], dtype=np.int64)
    w1T = np.ascontiguousarray(W1.T)
    b1c = np.ascontiguousarray(b1.reshape(H_T, 128).T)
    return x, W1, b1, We, be, eidx, w1T, b1c


def _i4_mat():
    i4 = np.zeros((128, STRIPS), dtype=np.float32)
    for j in range(STRIPS):
        i4[32 * j:32 * (j + 1), j] = 1.0
    return i4


def _prep_core_v3(x_sh, e_sh, weT, be, w1q, b1c, i4, nsh):
    """One core's v3 input map. Returns (map, order) or None on overflow."""
    nblk = nsh // CHUNK
    groups = nblk // STRIPS
    order = np.argsort(e_sh, kind="stable")
    e_sorted = e_sh[order]

    web = np.zeros((HID, nblk * SLOTS), dtype=np.float32)
    beh = np.zeros((128, groups), dtype=np.float32)
    ohb = np.zeros((128, groups * CHUNK), dtype=np.float32)
    for b in range(nblk):
        be_blk = e_sorted[b * CHUNK:(b + 1) * CHUNK]
        experts, slot_of = np.unique(be_blk, return_inverse=True)
        if len(experts) > SLOTS_DMA:
            return None
        g, j = b // STRIPS, b % STRIPS
        web[:, b * SLOTS:b * SLOTS + len(experts)] = weT[:, experts]
        beh[32 * j:32 * j + len(experts), g] = be[experts, 0]
        ohb[32 * j + slot_of, g * CHUNK + np.arange(CHUNK)] = 1.0

    m = {
        "xT": np.ascontiguousarray(x_sh[order].T * SX).astype(_fp8_dt()),
        "w1T": w1q,
        "b1c": b1c * SH,
        "web": web.astype(_bf16_dt()),
        "beh": beh,
        "ohb": ohb.astype(_bf16_dt()),
        "i4": i4,
    }
    return m, order


def _unpermute_core_v3(yd, order, nsh):
    """Device output [STRIPS, groups*CHUNK] -> original token order [nsh]."""
    groups = (nsh // CHUNK) // STRIPS
    ys = np.ascontiguousarray(
        yd.reshape(STRIPS, groups, CHUNK).transpose(1, 0, 2)
    ).reshape(nsh)
    yc = np.empty(nsh, dtype=np.float32)
    yc[order] = ys
    return yc


def _host_prep_v3(x, W1, b1, We, be, num, c):
    """Per-core maps for v3 plus the per-core inverse permutations.

    Returns (in_maps, orders) or None if a block spans too many experts."""
    x, W1, b1, We, be, eidx, w1T, b1c = _common_prep(x, W1, b1, We, be, num, c)
    weT = We[:, 0, :].T / SH                             # [HID, E], descale
    w1q = np.ascontiguousarray(w1T * SW).astype(_fp8_dt())
    i4 = _i4_mat()

    in_maps, orders = [], []
    for i in range(NCORES):
        sl = slice(i * NSH, (i + 1) * NSH)
        r = _prep_core_v3(x[sl], eidx[sl], weT, be, w1q, b1c, i4, NSH)
        if r is None:
            return None
        in_maps.append(r[0])
        orders.append(r[1])
    return in_maps, orders


def _host_prep_v1(x, W1, b1, We, be, num, c):
    x, W1, b1, We, be, eidx, w1T, b1c = _common_prep(x, W1, b1, We, be, num, c)
    w1T = _tf32_round(w1T)
    weT = np.zeros((HID, EP), dtype=np.float32)
    weT[:, :E] = We[:, 0, :].T
    weT = _tf32_round(weT)
    bec = np.zeros((EP, 1), dtype=np.float32)
    bec[:E, 0] = be[:, 0]
    oh_full = np.zeros((EP, N), dtype=np.float32)
    oh_full[eidx, np.arange(N)] = 1.0

    in_maps = []
    for i in range(NCORES):
        sl = slice(i * NSH, (i + 1) * NSH)
        in_maps.append({
            "xT": _tf32_round(x[sl].T),
            "w1T": w1T,
            "b1c": b1c,
            "weT": weT,
            "bec": bec,
            "oh": np.ascontiguousarray(oh_full[:, sl]),
        })
    return in_maps


def kernel(x, W1, b1, We, be, num, c):
    global LAST_RESULTS
    from concourse.bass_utils import run_bass_kernel_spmd

    prep = _host_prep_v3(x, W1, b1, We, be, num, c)
    if prep is not None:
        in_maps, orders = prep
        nc = _get_nc(3, NSH)
        res = run_bass_kernel_spmd(
            nc, in_maps, core_ids=list(range(NCORES)), trace=TRACE,
        )
        LAST_RESULTS = res
        out = np.empty(N, dtype=np.float32)
        for i in range(NCORES):
            out[i * NSH:(i + 1) * NSH] = _unpermute_core_v3(
                res.results[i]["y"], orders[i], NSH)
        return out.reshape(N, 1)

    in_maps = _host_prep_v1(x, W1, b1, We, be, num, c)
    nc = _get_nc(1, NSH)
    res = run_bass_kernel_spmd(
        nc, in_maps, core_ids=list(range(NCORES)), trace=TRACE,
    )
    LAST_RESULTS = res
    out = np.concatenate([r["y"].reshape(NSH) for r in res.results])
    return out.reshape(N, 1).astype(np.float32)


# revision 20
# speedup vs baseline: 1.0645x; 1.0645x over previous
"""Trainium2 Bass kernel for nn_CMVNet (moe_routing).

Reference computation:
    h = relu(x @ W1.T + b1)            # [N, HID]
    e = c[num]                         # [N] per-sample expert index
    y = einsum('noh,nh->no', We[e], h) + be[e]   # OUT=1
    out = sigmoid(y)                   # [N, 1]

Data-parallel over 8 cores (N/8 = 16384 rows each). Two device programs:

v3 (default): host sorts each shard's tokens by expert. Stage A runs in
  fp8 (e4m3) with perf_mode=DoubleRow: one matmul contracts all 256 input
  features (2 fp8 weights per PE cell), ~1.8x the bf16 rate. Host feeds
  x*16 and W1*16 in e4m3 so PSUM holds 256*(x@W1.T); the relu drain adds
  256*b1 and writes h*256 in bf16. Chunks are processed in PAIRS sharing
  a 2-bank [128, 1024] PSUM tile so each relu/bias op covers 1024 columns
  (amortizes the fixed ACT/DVE overhead); ops are split ACT/DVE to
  balance the two engines (both run ~1 elem/lane/cycle from fp32 PSUM).
  Stage B (per-sample expert dot) stays bf16: each 512-token block of the
  sorted order spans <= 8 distinct experts; 4 blocks are computed
  CONCURRENTLY (one per 32-column strip of the PE array via
  tile_position) against per-block expert slot tables (We/256 in bf16).
      select:  masked = (ps + beB) * ohB  (one fused DVE op),
               y4 = I4.T @ masked   ([4, 512] matvec on TensorE),
               sigmoid on ACT, DMA out, host un-permutes.
v1 (fallback, if any block spans > 8 experts): dense float32r scores
  against all 100 experts + one-hot select.

Quantization error (vs fp32 reference) is ~1.4e-2 relative L2, within
the 2e-2 gate; PSUM accumulation is fp32 throughout.
"""

import numpy as np

N, D_IN, HID, OUT, E = 131072, 256, 1024, 1, 100
NCORES = 8
NSH = N // NCORES          # 16384 rows per core
EP = 128                   # experts padded to full partition dim (v1)
CHUNK = 512
D_T = D_IN // 128          # 2 contraction k-tiles for stage A
H_T = HID // 128           # 8 hid tiles
NBLK = NSH // CHUNK        # 32 sorted-token blocks per core
STRIPS = 4                 # concurrent col-strips on the PE array
SLOTS = 32                 # on-chip expert slots per block
SLOTS_DMA = 8              # slots actually stored/DMAd per block
GROUPS = NBLK // STRIPS    # 8 strip-groups per core
SX = 16.0                  # fp8 input scale (x*SX in e4m3)
SW = 16.0                  # fp8 weight scale (W1*SW in e4m3)
SH = SX * SW               # h leaves stage A scaled by SH (bf16)

TRACE = False              # set by test harness for profiled runs
LAST_RESULTS = None        # BassKernelResults of the last run (for test.py)

_BUILT = {}                # (version, nsh) -> compiled Bass module


def _bf16_dt():
    import ml_dtypes
    return ml_dtypes.bfloat16


def _fp8_dt():
    import ml_dtypes
    return ml_dtypes.float8_e4m3   # TRN FP8_EXP4 (IEEE-ish, max +-240)


def _tf32_round(a):
    """Round fp32 ndarray to TF32 (10-bit mantissa), round-to-nearest-even."""
    u = np.ascontiguousarray(a, dtype=np.float32).view(np.uint32)
    u = (u + 0x00000FFF + ((u >> 13) & 1)) & np.uint32(0xFFFFE000)
    return u.view(np.float32)


def _mk_bass(nsh):
    from concourse import bacc
    return bacc.Bacc("TRN2", target_bir_lowering=False, debug=False)


# --------------------------------------------------------------------------
# v1: dense scores against all experts + one-hot select (fallback)
# --------------------------------------------------------------------------
def _build_nc_v1(nsh):
    from contextlib import ExitStack

    import concourse.mybir as mybir
    import concourse.tile as tile

    fp32 = mybir.dt.float32
    fr = mybir.dt.float32r
    AF = mybir.ActivationFunctionType
    OP = mybir.AluOpType

    nchunk = nsh // CHUNK
    nc = _mk_bass(nsh)

    xT = nc.dram_tensor("xT", [D_IN, nsh], fr, kind="ExternalInput")
    w1T = nc.dram_tensor("w1T", [D_IN, HID], fr, kind="ExternalInput")
    b1c = nc.dram_tensor("b1c", [128, H_T], fp32, kind="ExternalInput")
    weT = nc.dram_tensor("weT", [HID, EP], fr, kind="ExternalInput")
    bec = nc.dram_tensor("bec", [EP, 1], fp32, kind="ExternalInput")
    oh = nc.dram_tensor("oh", [EP, nsh], fp32, kind="ExternalInput")
    y = nc.dram_tensor("y", [1, nsh], fp32, kind="ExternalOutput")

    xT_v = xT.rearrange("(c p) n -> p c n", p=128)
    w1T_v = w1T.rearrange("(c p) h -> p c h", p=128)
    weT_v = weT.rearrange("(k p) e -> p k e", p=128)

    with tile.TileContext(nc) as tc, ExitStack() as ctx:
        cpool = ctx.enter_context(tc.tile_pool(name="consts", bufs=1))
        xin = ctx.enter_context(tc.tile_pool(name="xin", bufs=3))
        ohin = ctx.enter_context(tc.tile_pool(name="ohin", bufs=3))
        hpool = ctx.enter_context(tc.tile_pool(name="h", bufs=2))
        mpool = ctx.enter_context(tc.tile_pool(name="masked", bufs=3))
        ypool = ctx.enter_context(tc.tile_pool(name="yrow", bufs=3))
        php = ctx.enter_context(tc.tile_pool(name="ph", bufs=3, space="PSUM"))
        psp = ctx.enter_context(tc.tile_pool(name="ps", bufs=2, space="PSUM"))
        pyp = ctx.enter_context(tc.tile_pool(name="py", bufs=2, space="PSUM"))

        w1_sb = cpool.tile([128, D_T, HID], fr)
        we_sb = cpool.tile([128, H_T, EP], fr)
        b1_sb = cpool.tile([128, H_T], fp32)
        be_sb = cpool.tile([128, 1], fp32)
        ones_f32 = cpool.tile([128, 1], fp32)
        ones_sb = cpool.tile([128, 1], fr)
        nc.sync.dma_start(w1_sb[:], w1T_v[:])
        nc.sync.dma_start(we_sb[:], weT_v[:])
        nc.sync.dma_start(b1_sb[:], b1c[:])
        nc.sync.dma_start(be_sb[:], bec[:])
        nc.vector.memset(ones_f32[:], 1.0)
        nc.vector.tensor_copy(ones_sb[:], ones_f32[:])

        for ci in range(nchunk):
            n0 = ci * CHUNK
            xts = xin.tile([128, D_T, CHUNK], fr)
            nc.sync.dma_start(xts[:], xT_v[:, :, n0:n0 + CHUNK])
            oh_sb = ohin.tile([128, CHUNK], fp32)
            nc.sync.dma_start(oh_sb[:], oh[:, n0:n0 + CHUNK])

            hT = hpool.tile([128, H_T, CHUNK], fr)
            phs = []
            for j in range(H_T):
                ph = php.tile([128, CHUNK], fp32)
                for c in range(D_T):
                    nc.tensor.matmul(
                        ph[:],
                        w1_sb[:, c, 128 * j:128 * (j + 1)],
                        xts[:, c, :],
                        start=(c == 0),
                        stop=(c == D_T - 1),
                    )
                phs.append(ph)
            for j in range(H_T):
                if j % 2 == 0:
                    nc.scalar.activation(
                        hT[:, j, :], phs[j][:], AF.Relu,
                        bias=b1_sb[:, j:j + 1], scale=1.0,
                    )
                else:
                    nc.vector.tensor_scalar(
                        hT[:, j, :], phs[j][:],
                        b1_sb[:, j:j + 1], 0.0,
                        OP.add, OP.max,
                    )

            ps = psp.tile([128, CHUNK], fp32)
            for j in range(H_T):
                nc.tensor.matmul(
                    ps[:],
                    we_sb[:, j, :],
                    hT[:, j, :],
                    start=(j == 0),
                    stop=(j == H_T - 1),
                )

            masked = mpool.tile([128, CHUNK], fr)
            nc.vector.scalar_tensor_tensor(
                masked[:], ps[:], be_sb[:, 0:1], oh_sb[:],
                OP.add, OP.mult,
            )

            py = pyp.tile([1, CHUNK], fp32)
            nc.tensor.matmul(py[:], ones_sb[:], masked[:], start=True, stop=True)
            y_sb = ypool.tile([1, CHUNK], fp32)
            nc.scalar.activation(y_sb[:], py[:], AF.Sigmoid)
            nc.sync.dma_start(y[0:1, n0:n0 + CHUNK], y_sb[:])

    nc.compile()
    return nc


# --------------------------------------------------------------------------
# v3: fp8 DoubleRow stage A, paired-chunk wide relu, bf16 strip stage B
# --------------------------------------------------------------------------
def _build_nc_v3(nsh):
    from contextlib import ExitStack

    import concourse.mybir as mybir
    import concourse.tile as tile

    fp32 = mybir.dt.float32
    fr = mybir.dt.float32r
    bf16 = mybir.dt.bfloat16
    f8 = mybir.dt.float8e4
    AF = mybir.ActivationFunctionType
    OP = mybir.AluOpType
    DR = mybir.MatmulPerfMode.DoubleRow

    nblk = nsh // CHUNK
    groups = nblk // STRIPS
    PAIR = 2 * CHUNK
    nc = _mk_bass(nsh)

    xT = nc.dram_tensor("xT", [D_IN, nsh], f8, kind="ExternalInput")
    w1T = nc.dram_tensor("w1T", [D_IN, HID], f8, kind="ExternalInput")
    b1c = nc.dram_tensor("b1c", [128, H_T], fp32, kind="ExternalInput")
    web = nc.dram_tensor("web", [HID, nblk * SLOTS], bf16,
                         kind="ExternalInput")
    beh = nc.dram_tensor("beh", [128, groups], fp32, kind="ExternalInput")
    ohb = nc.dram_tensor("ohb", [128, groups * CHUNK], bf16,
                         kind="ExternalInput")
    i4 = nc.dram_tensor("i4", [128, STRIPS], fr, kind="ExternalInput")
    y = nc.dram_tensor("y", [STRIPS, groups * CHUNK], fp32,
                       kind="ExternalOutput")

    xT_v = xT.rearrange("(c p) n -> p c n", p=128)
    w1T_v = w1T.rearrange("(c p) h -> p c h", p=128)
    web_v = web.rearrange("(k p) (b s) -> p k b s", p=128, s=SLOTS)

    with tile.TileContext(nc) as tc, ExitStack() as ctx:
        cpool = ctx.enter_context(tc.tile_pool(name="consts", bufs=1))
        xin = ctx.enter_context(tc.tile_pool(name="xin", bufs=4))
        ohin = ctx.enter_context(tc.tile_pool(name="ohin", bufs=2))
        hpool = ctx.enter_context(tc.tile_pool(name="h", bufs=4))
        mpool = ctx.enter_context(tc.tile_pool(name="masked", bufs=2))
        ypool = ctx.enter_context(tc.tile_pool(name="yrow", bufs=2))
        # ALL of PSUM is one pool: 4 slots x 2 banks. Stage-A fp32 tiles
        # rotate through it; the stage-B score tile, the select output and
        # the warmup target borrow slots transiently. 4-deep buffering is
        # what keeps the relu drains (the bottleneck engines) saturated
        # across the ~100ns semaphore hops of the fill->drain->refill loop.
        php = ctx.enter_context(tc.tile_pool(name="ph", bufs=4, space="PSUM"))

        w1_sb = cpool.tile([128, D_T, HID], f8)
        web_sb = cpool.tile([128, H_T, nblk, SLOTS], bf16)
        b1_sb = cpool.tile([128, H_T], fp32)
        beh_sb = cpool.tile([128, groups], fp32)
        i4_sb = cpool.tile([128, STRIPS], fr)
        # Cold DMA completions serialize at ~2us apiece per queue, so the
        # head spreads the gating transfers across the sync and gpsimd
        # queues (never the ACT/scalar queue - DMA issue there delays the
        # first relus). x half 0 goes FIRST on sync; w1a/b1 + x half 1
        # lead the gpsimd queue ahead of the bulk slot table.
        nc.gpsimd.dma_start(w1_sb[:, :, 0:128], w1T_v[:, :, 0:128])
        nc.gpsimd.dma_start(b1_sb[:], b1c[:])
        # warm BOTH ACT tables during the idle head (each activation
        # function's first use otherwise pays a 1.3us mid-stream
        # ACT_TABLE_LOAD stall)
        warm_sb = cpool.tile([1, 2], fp32)
        nc.vector.memset(warm_sb[:], 0.0)
        nc.scalar.activation(warm_sb[:], warm_sb[:], AF.Sigmoid)
        nc.scalar.activation(warm_sb[:], warm_sb[:], AF.Relu)
        # warm the PE HAM clock gate during the DMA head with FULL-ARRAY
        # zero matmuls (no DMA dependency). Tiny 8-col warmups give almost
        # no cycle credit and leave the PE at the MID p-state (630ns DR
        # matmuls) deep into the kernel; [128x128]x512 ones accumulate
        # real busy cycles. Sized to end roughly when the first x lands.
        wmm = cpool.tile([128, CHUNK], bf16)
        nc.vector.memset(wmm[:], 0.0)
        wps = php.tile([128, CHUNK], fp32, tag="ph")
        for _ in range(10):
            nc.tensor.matmul(wps[:], wmm[:, 0:128], wmm[:],
                             start=True, stop=True)

        def emit_select(sg, smasked, split=1):
            # y4 = I4.T @ masked, sigmoid, DMA out. Emitted one group late
            # so the PE never waits on the DVE mask op.
            w = CHUNK // split
            for hh in range(split):
                h0 = hh * w
                py = php.tile([STRIPS, w], fp32, tag="ph")
                nc.tensor.matmul(py[:], i4_sb[:], smasked[:, h0:h0 + w],
                                 start=True, stop=True)
                y_sb = ypool.tile([STRIPS, w], fp32, tag="ysb")
                nc.scalar.activation(y_sb[:], py[:], AF.Sigmoid)
                nc.sync.dma_start(
                    y[:, sg * CHUNK + h0:sg * CHUNK + h0 + w], y_sb[:])

        pending_select = None
        for g in range(groups):
            hts = []
            for lp in range(2):
                p = 2 * g + lp
                n0 = p * PAIR
                xts = xin.tile([128, D_T, PAIR], f8, tag="xts")
                if p == 0:
                    # first pair split across two queues so both halves
                    # complete their (serial, ~2us) cold DMA in parallel
                    nc.sync.dma_start(xts[:, :, 0:CHUNK],
                                      xT_v[:, :, n0:n0 + CHUNK])
                    nc.gpsimd.dma_start(xts[:, :, CHUNK:PAIR],
                                        xT_v[:, :, n0 + CHUNK:n0 + PAIR])
                    nc.gpsimd.dma_start(w1_sb[:, :, 128:HID],
                                        w1T_v[:, :, 128:HID])
                    # bulk slot table + small consts follow on gpsimd
                    nc.gpsimd.dma_start(web_sb[:], web_v[:])
                    nc.gpsimd.dma_start(beh_sb[:], beh[:])
                    nc.gpsimd.dma_start(i4_sb[:], i4[:])
                else:
                    nc.sync.dma_start(xts[:], xT_v[:, :, n0:n0 + PAIR])
                if lp == 0:
                    # prefetch the one-hot select mask for this group
                    oh_sb = ohin.tile([128, CHUNK], bf16)
                    nc.sync.dma_start(
                        oh_sb[:], ohb[:, g * CHUNK:(g + 1) * CHUNK])

                hT = hpool.tile([128, H_T, PAIR], bf16, tag="hT")
                for k in range(H_T):
                    ph = php.tile([128, PAIR], fp32, tag="ph")
                    for hh in range(2):
                        nc.tensor.matmul(
                            ph[:, hh * CHUNK:(hh + 1) * CHUNK],
                            w1_sb[:, :, 128 * k:128 * (k + 1)],
                            xts[:, :, hh * CHUNK:(hh + 1) * CHUNK],
                            start=True,
                            stop=True,
                            perf_mode=DR,
                        )
                    # relu+bias drain, split ACT/DVE ~4.25/3.75 (measured:
                    # ACT ~1.12us, DVE ~1.28us per 1024-wide op; DVE also
                    # owns the mask op, ACT the sigmoids)
                    on_act = (k % 2 == 0) or (
                        k == 7 and lp == 1 and g % 2 == 0)
                    if on_act:
                        nc.scalar.activation(
                            hT[:, k, :], ph[:], AF.Relu,
                            bias=b1_sb[:, k:k + 1], scale=1.0,
                        )
                    else:
                        nc.vector.tensor_scalar(
                            hT[:, k, :], ph[:],
                            b1_sb[:, k:k + 1], 0.0,
                            OP.add, OP.max,
                        )
                    if lp == 1 and k == 0 and pending_select is not None:
                        emit_select(*pending_select)
                        pending_select = None
                hts.append(hT)

            # stage B: 4 blocks concurrently, one per 32-col strip (bf16);
            # the score tile borrows a rotating PSUM slot. While the PE
            # runs these 8 k-steps the drains chew through their 4-deep
            # backlog, so they never idle.
            ps = php.tile([128, CHUNK], fp32, tag="ph")
            for k in range(H_T):
                for j in range(STRIPS):
                    blk = g * STRIPS + j
                    pair, off = j // 2, (j % 2) * CHUNK
                    nc.tensor.matmul(
                        ps[32 * j:32 * (j + 1), :],
                        web_sb[:, k, blk, :],
                        hts[pair][:, k, off:off + CHUNK],
                        start=(k == 0),
                        stop=(k == H_T - 1),
                        tile_position=(0, 32 * j),
                        skip_group_check=True,
                    )
            masked = mpool.tile([128, CHUNK], fr)
            nc.vector.scalar_tensor_tensor(
                masked[:], ps[:], beh_sb[:, g:g + 1], oh_sb[:],
                OP.add, OP.mult,
            )
            if g == groups - 1:
                # final select split in halves so DVE/PE/ACT/DMA pipeline
                # in the kernel tail instead of serializing full-width
                emit_select(g, masked, split=2)
            else:
                pending_select = (g, masked)

    nc.compile()
    return nc


def _get_nc(version, nsh=NSH):
    key = (version, nsh)
    if key not in _BUILT:
        _BUILT[key] = (_build_nc_v3 if version == 3 else _build_nc_v1)(nsh)
    return _BUILT[key]


# --------------------------------------------------------------------------
# host prep
# --------------------------------------------------------------------------
def _common_prep(x, W1, b1, We, be, num, c):
    x = np.asarray(x, dtype=np.float32)
    W1 = np.asarray(W1, dtype=np.float32)
    b1 = np.asarray(b1, dtype=np.float32)
    We = np.asarray(We, dtype=np.float32)
    be = np.asarray(be, dtype=np.float32)
    eidx = np.asarray(np.asarray(c)[np.asarray(num)], dtype=np.int64)
    w1T = np.ascontiguousarray(W1.T)
    b1c = np.ascontiguousarray(b1.reshape(H_T, 128).T)
    return x, W1, b1, We, be, eidx, w1T, b1c


def _i4_mat():
    i4 = np.zeros((128, STRIPS), dtype=np.float32)
    for j in range(STRIPS):
        i4[32 * j:32 * (j + 1), j] = 1.0
    return i4


def _prep_core_v3(x_sh, e_sh, weT, be, w1q, b1c, i4, nsh):
    """One core's v3 input map. Returns (map, order) or None on overflow."""
    nblk = nsh // CHUNK
    groups = nblk // STRIPS
    order = np.argsort(e_sh, kind="stable")
    e_sorted = e_sh[order]

    web = np.zeros((HID, nblk * SLOTS), dtype=np.float32)
    beh = np.zeros((128, groups), dtype=np.float32)
    ohb = np.zeros((128, groups * CHUNK), dtype=np.float32)
    for b in range(nblk):
        be_blk = e_sorted[b * CHUNK:(b + 1) * CHUNK]
        experts, slot_of = np.unique(be_blk, return_inverse=True)
        if len(experts) > SLOTS_DMA:
            return None
        g, j = b // STRIPS, b % STRIPS
        web[:, b * SLOTS:b * SLOTS + len(experts)] = weT[:, experts]
        beh[32 * j:32 * j + len(experts), g] = be[experts, 0]
        ohb[32 * j + slot_of, g * CHUNK + np.arange(CHUNK)] = 1.0

    m = {
        "xT": np.ascontiguousarray(x_sh[order].T * SX).astype(_fp8_dt()),
        "w1T": w1q,
        "b1c": b1c * SH,
        "web": web.astype(_bf16_dt()),
        "beh": beh,
        "ohb": ohb.astype(_bf16_dt()),
        "i4": i4,
    }
    return m, order


def _unpermute_core_v3(yd, order, nsh):
    """Device output [STRIPS, groups*CHUNK] -> original token order [nsh]."""
    groups = (nsh // CHUNK) // STRIPS
    ys = np.ascontiguousarray(
        yd.reshape(STRIPS, groups, CHUNK).transpose(1, 0, 2)
    ).reshape(nsh)
    yc = np.empty(nsh, dtype=np.float32)
    yc[order] = ys
    return yc


def _host_prep_v3(x, W1, b1, We, be, num, c):
    """Per-core maps for v3 plus the per-core inverse permutations.

    Returns (in_maps, orders) or None if a block spans too many experts."""
    x, W1, b1, We, be, eidx, w1T, b1c = _common_prep(x, W1, b1, We, be, num, c)
    weT = We[:, 0, :].T / SH                             # [HID, E], descale
    w1q = np.ascontiguousarray(w1T * SW).astype(_fp8_dt())
    i4 = _i4_mat()

    in_maps, orders = [], []
    for i in range(NCORES):
        sl = slice(i * NSH, (i + 1) * NSH)
        r = _prep_core_v3(x[sl], eidx[sl], weT, be, w1q, b1c, i4, NSH)
        if r is None:
            return None
        in_maps.append(r[0])
        orders.append(r[1])
    return in_maps, orders


def _host_prep_v1(x, W1, b1, We, be, num, c):
    x, W1, b1, We, be, eidx, w1T, b1c = _common_prep(x, W1, b1, We, be, num, c)
    w1T = _tf32_round(w1T)
    weT = np.zeros((HID, EP), dtype=np.float32)
    weT[:, :E] = We[:, 0, :].T
    weT = _tf32_round(weT)
    bec = np.zeros((EP, 1), dtype=np.float32)
    bec[:E, 0] = be[:, 0]
    oh_full = np.zeros((EP, N), dtype=np.float32)
    oh_full[eidx, np.arange(N)] = 1.0

    in_maps = []
    for i in range(NCORES):
        sl = slice(i * NSH, (i + 1) * NSH)
        in_maps.append({
            "xT": _tf32_round(x[sl].T),
            "w1T": w1T,
            "b1c": b1c,
            "weT": weT,
            "bec": bec,
            "oh": np.ascontiguousarray(oh_full[:, sl]),
        })
    return in_maps


def kernel(x, W1, b1, We, be, num, c):
    global LAST_RESULTS
    from concourse.bass_utils import run_bass_kernel_spmd

    prep = _host_prep_v3(x, W1, b1, We, be, num, c)
    if prep is not None:
        in_maps, orders = prep
        nc = _get_nc(3, NSH)
        res = run_bass_kernel_spmd(
            nc, in_maps, core_ids=list(range(NCORES)), trace=TRACE,
        )
        LAST_RESULTS = res
        out = np.empty(N, dtype=np.float32)
        for i in range(NCORES):
            out[i * NSH:(i + 1) * NSH] = _unpermute_core_v3(
                res.results[i]["y"], orders[i], NSH)
        return out.reshape(N, 1)

    in_maps = _host_prep_v1(x, W1, b1, We, be, num, c)
    nc = _get_nc(1, NSH)
    res = run_bass_kernel_spmd(
        nc, in_maps, core_ids=list(range(NCORES)), trace=TRACE,
    )
    LAST_RESULTS = res
    out = np.concatenate([r["y"].reshape(NSH) for r in res.results])
    return out.reshape(N, 1).astype(np.float32)
